# revision 15
# baseline (speedup 1.0000x reference)
"""Trainium2 Bass kernel for nn_MultiHeadAttention_39582418600023.

Model (reference bug preserved: Q = K = V = x @ W_Q):
  qkv = x @ W_Q; q,k,v = heads(qkv)
  out = softmax(causal(q k^T) / sqrt(dh)) v  ->  ctx @ W_out + b_out

Sharding (8 cores): data-parallel over batch (4) x tensor-parallel over
head groups (2).  Core c handles batch c//2, heads (c%2)*8 .. +8
(W_Q column-parallel, W_out row-parallel); host sums the two partial
out-projections per batch and adds the bias.

Per-core device kernel (engine budget: ACT=exp only ~150us, PE ~220us,
DVE evictions/masks/norms ~130us):
  1. qkvT[e,t] = W_Qc^T @ x^T (fp32r, x streamed in double-buffered
     512-col chunks over two DMA queues); V via PE transposes into
     ones-augmented VA.
  2. Attention+output fused in one pool block, th(=2 qc)-outer:
     scores for a head pair land in one [128,1024] PSUM tile (ring
     shared with the out-projection PSUM), one exp-ACTIVATE per
     (pair,qc,kb) writes bf16 probs consumed by PV matmuls (VA^T@PT,
     row 64 = softmax denominator).  Causal mask = bf16 upper-tri
     multiply on DVE.  Denominator rows DMA into DST[16,512]; one
     batched reciprocal per th; K=16 one-hot selector matmuls
     broadcast 1/d across partitions; DVE normalizes ctx in place.
  3. Out-projection for a th runs right after its norm, overlapping
     the next th's attention: W_out block stationary (LDWEIGHTS
     reuse), CTXT moving, emits transposed [D,S] partials (host
     transposes back).
"""
import os
import sys

sys.path.insert(0, "/opt/trn_rl_repo")
os.environ.setdefault("MYCRO_LOCAL_CACHE", "1")

import numpy as np

B, S, D = 4, 2048, 1024
NH, DH = 16, 64
EH = 512          # e-columns per core (8 local heads)
NHL = 8           # local heads
N_CORES = 8

_CACHE = {}


def _build():
    import concourse.mybir as mybir
    import concourse.tile as tile
    from concourse import bacc
    from concourse.masks import make_identity, make_upper_triangular

    F32 = mybir.dt.float32
    F32R = mybir.dt.float32r
    BF16 = mybir.dt.bfloat16
    EXP = mybir.ActivationFunctionType.Exp

    nc = bacc.Bacc(None, target_bir_lowering=False, debug=True)
    with tile.TileContext(nc) as tc:
        with tc.tile_pool(name="dram", bufs=1, space="DRAM") as dram:
            xT = dram.tile([D, S], F32, kind="ExternalInput")      # x[b].T
            wq = dram.tile([D, EH], F32, kind="ExternalInput")     # W_Q cols
            wo = dram.tile([EH, D], F32, kind="ExternalInput")     # W_out rows
            # partial out, TRANSPOSED [D, S]; host transposes back
            outp = dram.tile([D, S], F32, kind="ExternalOutput")

            with tc.tile_pool(name="persist", bufs=1) as pp:
                # qkvT: [e-block 128, eb, t], bf16 (scores operands)
                QKVT = pp.tile([128, 4, S], BF16)
                # ones-augmented V (bf16): [t%128, tb, h, 0:64]=V, [..,64]=1
                VA = pp.tile([128, 16, NHL, DH + 1], BF16)
                # unnormalized ctxT (bf16), same layout as QKVT
                CTXT = pp.tile([128, 4, S], BF16)
                IDN = pp.tile([128, 128], BF16)
                MASKF = pp.tile([128, 128], F32)  # 1 on i<=j else 0
                MASKB = pp.tile([128, 2, 128], BF16)
                make_upper_triangular(nc, MASKF[:], val=1.0, diag=True)
                make_identity(nc, IDN[:])
                nc.vector.tensor_copy(MASKB[:, 0, :], MASKF[:])
                nc.vector.tensor_copy(MASKB[:, 1, :], MASKF[:])
                nc.vector.memset(VA[:, :, :, DH : DH + 1], 1.0)
                # denominator staging per th (2 qc x 4 jb x 2 hh rows):
                # row r = (qc%2)*8 + jb*2 + hh, partitions 0..15
                DST = [pp.tile([16, 512], F32, name=f"DST{t}")
                       for t in range(2)]
                RST = [pp.tile([16, 512], F32R, name=f"RST{t}")
                       for t in range(2)]
                # one-hot selectors [16, pair, 128]: SEL[k, p, j] = 1 iff
                # k == (qc%2)*8 + jb*2 + (j>=64); BC = SEL^T @ RST
                SELF_ = pp.tile([16, 8, 128], F32)
                SEL = [pp.tile([16, 8, 128], F32R, name=f"SEL{t}")
                       for t in range(2)]
                nc.gpsimd.memset(SELF_[:], 1.0)
                for qh in range(2):
                    for jb in range(4):
                        pr = qh * 4 + jb
                        for hh in range(2):
                            ks = qh * 8 + jb * 2 + hh
                            nc.gpsimd.affine_select(
                                out=SELF_[:, pr, hh * 64 : hh * 64 + 64],
                                in_=SELF_[:, pr, hh * 64 : hh * 64 + 64],
                                compare_op=mybir.AluOpType.is_equal,
                                fill=0.0, base=-ks,
                                pattern=[[0, 64]], channel_multiplier=1)
                nc.vector.tensor_copy(SEL[0][:], SELF_[:])
                nc.vector.tensor_copy(SEL[1][:], SELF_[:])
                # denom rows staged at partition 64 (DVE evict from PSUM;
                # DMA cannot read PSUM)
                DROW0 = pp.tile([65, 2, 512], F32)
                DROW1 = pp.tile([65, 2, 512], F32)
                # out-proj weights (bf16), loaded on the SW DGE queue so
                # the HW DGE queues stay free for x^T chunks
                WO = pp.tile([128, 4, D], BF16)
                WOF = pp.tile([128, 4, D], F32)
                for eb in range(4):
                    nc.gpsimd.dma_start(
                        out=WOF[:, eb, :],
                        in_=wo[eb * 128 : (eb + 1) * 128, :])
                    nc.vector.tensor_copy(WO[:, eb, :], WOF[:, eb, :])

                # ------------ phase 1: projection + V transposes ------
                with tc.tile_pool(name="px", bufs=2) as px, \
                     tc.tile_pool(name="pwq", bufs=1) as pwq, \
                     tc.tile_pool(name="pj", bufs=4, space="PSUM") as pj, \
                     tc.tile_pool(name="ptr", bufs=4, space="PSUM") as ptr:
                    WQ = pwq.tile([128, 8, EH], F32R)
                    for kc in range(8):
                        nc.sync.dma_start(
                            out=WQ[:, kc, :],
                            in_=wq[kc * 128 : (kc + 1) * 128, :].bitcast(F32R))
                    # x^T streamed by 512-query column chunks (tn-major),
                    # alternating kc between two DMA queues
                    for tn in range(4):
                        XTC = px.tile([128, 8, 512], F32R, tag="xtc")
                        for kc in range(8):
                            q = nc.scalar if kc % 2 else nc.sync
                            q.dma_start(
                                out=XTC[:, kc, :],
                                in_=xT[kc * 128 : (kc + 1) * 128,
                                       tn * 512 : (tn + 1) * 512
                                       ].bitcast(F32R))
                        for eb in range(4):
                            ps = pj.tile([128, 512], F32, tag="pj")
                            for kc in range(8):
                                nc.tensor.matmul(
                                    ps[:],
                                    WQ[:, kc, eb * 128 : (eb + 1) * 128],
                                    XTC[:, kc, :],
                                    start=(kc == 0), stop=(kc == 7))
                            nc.vector.tensor_copy(
                                QKVT[:, eb, tn * 512 : (tn + 1) * 512],
                                ps[:])
                        # V = transposed qkvT blocks for these 4 t-blocks
                        for jb in range(4):
                            for tb in range(4 * tn, 4 * tn + 4):
                                tp = ptr.tile([128, 128], BF16, tag="tp")
                                nc.tensor.transpose(
                                    tp[:],
                                    QKVT[:, jb, tb * 128 : (tb + 1) * 128],
                                    IDN[:])
                                nc.vector.tensor_copy(
                                    VA[:, tb, 2 * jb : 2 * jb + 2, 0:DH],
                                    tp[:].rearrange(
                                        "p (h d) -> p h d", h=2))

                # ------- phase 2+3 fused: attention + norm + out-proj --
                # PSUM rings: pr1 [128,1024] x2 (scores & out-proj),
                # pr2 [128,512] x4 (PV accumulators & 1/d broadcasts)
                with tc.tile_pool(name="ptp", bufs=14) as ptp, \
                     tc.tile_pool(name="po", bufs=3) as po, \
                     tc.tile_pool(name="pr1", bufs=2, space="PSUM") as pr1, \
                     tc.tile_pool(name="pr2", bufs=4, space="PSUM") as pr2:
                    for th in range(2):
                        for qc in (2 * th, 2 * th + 1):
                            qs = qc * 512
                            nkb = 4 * qc + 4
                            for jb in range(4):
                                qA = QKVT[0:64, jb, :]    # head 2jb
                                qB = QKVT[64:128, jb, :]  # head 2jb+1
                                CA = pr2.tile([128, 512], F32, tag="r2")
                                CB = pr2.tile([128, 512], F32, tag="r2")
                                pts = []
                                for kb in range(nkb):
                                    k0 = kb * 128
                                    q0 = max(k0, qs)
                                    n = qs + 512 - q0
                                    po_ = q0 - qs
                                    sc = pr1.tile([128, 1024], F32,
                                                  tag="r1")
                                    nc.tensor.matmul(
                                        sc[:, 0:n],
                                        qA[:, k0 : k0 + 128],
                                        qA[:, q0 : q0 + n],
                                        start=True, stop=True)
                                    nc.tensor.matmul(
                                        sc[:, 512 : 512 + n],
                                        qB[:, k0 : k0 + 128],
                                        qB[:, q0 : q0 + n],
                                        start=True, stop=True)
                                    pt = ptp.tile([128, 2, n], BF16,
                                                  tag="pt")
                                    nc.scalar.activation(
                                        pt[:],
                                        sc[:].rearrange(
                                            "p (two n) -> p two n", two=2)[
                                            :, :, 0:n],
                                        EXP, scale=0.125)
                                    if k0 >= qs:  # diagonal 128x128 block
                                        nc.vector.tensor_mul(
                                            pt[:, :, 0:128],
                                            pt[:, :, 0:128], MASKB[:])
                                    pts.append((pt, po_, n))
                                for hh, C in ((0, CA), (1, CB)):
                                    for kb, (pt, po_, n) in enumerate(pts):
                                        nc.tensor.matmul(
                                            C[0:65, po_ : po_ + n],
                                            VA[:, kb, 2 * jb + hh, :],
                                            pt[:, hh, :],
                                            start=(kb == 0),
                                            stop=(kb == nkb - 1))
                                for hh, C in ((0, CA), (1, CB)):
                                    nc.vector.tensor_copy(
                                        CTXT[hh * 64 : hh * 64 + 64, jb,
                                             qs : qs + 512],
                                        C[0:64, :])
                                    DR = DROW0 if hh == 0 else DROW1
                                    nc.vector.tensor_copy(
                                        DR[64:65, qc % 2, :], C[64:65, :])
                                    r = (qc % 2) * 8 + jb * 2 + hh
                                    nc.sync.dma_start(
                                        out=DST[th][r : r + 1, :],
                                        in_=DR[64:65, qc % 2, :])
                        with nc.allow_low_precision(
                                reason="f32r recip 1e-4 ok"):
                            nc.vector.reciprocal(RST[th][:], DST[th][:])
                        for qc in (2 * th, 2 * th + 1):
                            for jb in range(4):
                                pr_ = (qc % 2) * 4 + jb
                                BC = pr2.tile([128, 512], F32, tag="r2")
                                nc.tensor.matmul(
                                    BC[:], SEL[th][:, pr_, :],
                                    RST[th][:], start=True, stop=True)
                                for hh in range(2):
                                    dst = CTXT[hh * 64 : hh * 64 + 64, jb,
                                               qc * 512 : qc * 512 + 512]
                                    nc.vector.tensor_mul(
                                        dst, dst,
                                        BC[hh * 64 : hh * 64 + 64, :])
                        # out-projection for this th (overlaps next th's
                        # attention): W_out stationary, CTXT moving
                        for db in range(8):
                            ps = pr1.tile([128, 1024], F32, tag="r1")
                            for eb in range(4):
                                for tcc in range(2):
                                    t0 = (2 * th + tcc) * 512
                                    nc.tensor.matmul(
                                        ps[:, tcc * 512 :
                                           tcc * 512 + 512],
                                        WO[:, eb,
                                           db * 128 : (db + 1) * 128],
                                        CTXT[:, eb, t0 : t0 + 512],
                                        start=(eb == 0), stop=(eb == 3))
                            ob = po.tile([128, 1024], F32, tag="ob")
                            nc.vector.tensor_copy(ob[:], ps[:])
                            nc.sync.dma_start(
                                out=outp[db * 128 : (db + 1) * 128,
                                         th * 1024 : (th + 1) * 1024],
                                in_=ob[:])
    nc.compile()
    return nc, {"xT": xT.name, "wq": wq.name, "wo": wo.name,
                "outp": outp.name}


def _get():
    if "nc" not in _CACHE:
        _CACHE["nc"], _CACHE["names"] = _build()
    return _CACHE["nc"], _CACHE["names"]


def _run(x, W_Q, W_out, trace=False):
    from concourse.bass_utils import run_bass_kernel_spmd

    nc, nm = _get()
    in_maps = []
    for c in range(N_CORES):
        b, hg = c // 2, c % 2
        in_maps.append({
            nm["xT"]: np.ascontiguousarray(x[b].T.astype(np.float32)),
            nm["wq"]: np.ascontiguousarray(
                W_Q[:, hg * EH : (hg + 1) * EH].astype(np.float32)),
            nm["wo"]: np.ascontiguousarray(
                W_out[hg * EH : (hg + 1) * EH, :].astype(np.float32)),
        })
    return run_bass_kernel_spmd(
        nc, in_maps, list(range(N_CORES)), trace=trace), nm


def kernel(x, W_Q, W_out, b_out):
    res, nm = _run(np.asarray(x), np.asarray(W_Q), np.asarray(W_out))
    bo = np.asarray(b_out, dtype=np.float32)
    out = np.empty((B, S, D), np.float32)
    for b in range(B):
        # device emits [D, S] partials; transpose back on host
        out[b] = (res.results[2 * b][nm["outp"]]
                  + res.results[2 * b + 1][nm["outp"]]).T + bo
    return out


# revision 18
# speedup vs baseline: 1.0241x; 1.0241x over previous
"""Trainium2 Bass kernel for nn_MultiHeadAttention_39582418600023.

Model (reference bug preserved: Q = K = V = x @ W_Q):
  qkv = x @ W_Q; q,k,v = heads(qkv)
  out = softmax(causal(q k^T) / sqrt(dh)) v  ->  ctx @ W_out + b_out

Sharding (8 cores): data-parallel over batch (4) x tensor-parallel over
head groups (2).  Core c handles batch c//2, heads (c%2)*8 .. +8
(W_Q column-parallel, W_out row-parallel); host sums the two partial
out-projections per batch and adds the bias.

Per-core device kernel (engine budget: ACT=exp only ~150us, PE ~220us,
DVE evictions/masks/norms ~130us):
  1. qkvT[e,t] = W_Qc^T @ x^T (fp32r, x streamed in double-buffered
     512-col chunks over two DMA queues); V via PE transposes into
     ones-augmented VA.
  2. Attention+output fused in one pool block, th(=2 qc)-outer:
     scores for a head pair land in one [128,1024] PSUM tile (ring
     shared with the out-projection PSUM), one exp-ACTIVATE per
     (pair,qc,kb) writes bf16 probs consumed by PV matmuls (VA^T@PT,
     row 64 = softmax denominator).  Causal mask = bf16 upper-tri
     multiply on DVE.  Denominator rows DMA into DST[16,512]; one
     batched reciprocal per th; K=16 one-hot selector matmuls
     broadcast 1/d across partitions; DVE normalizes ctx in place.
  3. Out-projection for a th runs right after its norm, overlapping
     the next th's attention: W_out block stationary (LDWEIGHTS
     reuse), CTXT moving, emits transposed [D,S] partials (host
     transposes back).
"""
import os
import sys

sys.path.insert(0, "/opt/trn_rl_repo")
os.environ.setdefault("MYCRO_LOCAL_CACHE", "1")

import numpy as np

B, S, D = 4, 2048, 1024
NH, DH = 16, 64
EH = 512          # e-columns per core (8 local heads)
NHL = 8           # local heads
N_CORES = 8

_CACHE = {}


def _build():
    import concourse.mybir as mybir
    import concourse.tile as tile
    from concourse import bacc
    from concourse.masks import make_identity, make_upper_triangular

    F32 = mybir.dt.float32
    F32R = mybir.dt.float32r
    BF16 = mybir.dt.bfloat16
    EXP = mybir.ActivationFunctionType.Exp

    nc = bacc.Bacc(None, target_bir_lowering=False, debug=True)
    with tile.TileContext(nc) as tc:
        with tc.tile_pool(name="dram", bufs=1, space="DRAM") as dram:
            xT = dram.tile([D, S], F32, kind="ExternalInput")      # x[b].T
            wq = dram.tile([D, EH], F32, kind="ExternalInput")     # W_Q cols
            wo = dram.tile([EH, D], F32, kind="ExternalInput")     # W_out rows
            # partial out, TRANSPOSED [D, S]; host transposes back
            outp = dram.tile([D, S], F32, kind="ExternalOutput")

            with tc.tile_pool(name="persist", bufs=1) as pp:
                # qkvT: [e-block 128, eb, t], bf16 (scores operands)
                QKVT = pp.tile([128, 4, S], BF16)
                # ones-augmented V (bf16): [t%128, tb, h, 0:64]=V, [..,64]=1
                VA = pp.tile([128, 16, NHL, DH + 1], BF16)
                # unnormalized ctxT (bf16), same layout as QKVT
                CTXT = pp.tile([128, 4, S], BF16)
                IDN = pp.tile([128, 128], BF16)
                MASKF = pp.tile([128, 128], F32)  # 1 on i<=j else 0
                MASKB = pp.tile([128, 2, 128], BF16)
                make_upper_triangular(nc, MASKF[:], val=1.0, diag=True)
                make_identity(nc, IDN[:])
                nc.vector.tensor_copy(MASKB[:, 0, :], MASKF[:])
                nc.vector.tensor_copy(MASKB[:, 1, :], MASKF[:])
                nc.vector.memset(VA[:, :, :, DH : DH + 1], 1.0)
                # denominator staging per th (2 qc x 4 jb x 2 hh rows):
                # row r = (qc%2)*8 + jb*2 + hh, partitions 0..15
                DST = [pp.tile([16, 512], F32, name=f"DST{t}")
                       for t in range(2)]
                RST = [pp.tile([16, 512], F32R, name=f"RST{t}")
                       for t in range(2)]
                # one-hot selectors [16, pair, 128]: SEL[k, p, j] = 1 iff
                # k == (qc%2)*8 + jb*2 + (j>=64); BC = SEL^T @ RST
                SELF_ = pp.tile([16, 8, 128], F32)
                SEL = [pp.tile([16, 8, 128], F32R, name=f"SEL{t}")
                       for t in range(2)]
                nc.gpsimd.memset(SELF_[:], 1.0)
                for qh in range(2):
                    for jb in range(4):
                        pr = qh * 4 + jb
                        for hh in range(2):
                            ks = qh * 8 + jb * 2 + hh
                            nc.gpsimd.affine_select(
                                out=SELF_[:, pr, hh * 64 : hh * 64 + 64],
                                in_=SELF_[:, pr, hh * 64 : hh * 64 + 64],
                                compare_op=mybir.AluOpType.is_equal,
                                fill=0.0, base=-ks,
                                pattern=[[0, 64]], channel_multiplier=1)
                nc.vector.tensor_copy(SEL[0][:], SELF_[:])
                nc.vector.tensor_copy(SEL[1][:], SELF_[:])
                # denom rows staged at partition 64 (DVE evict from PSUM;
                # DMA cannot read PSUM)
                DROW0 = pp.tile([65, 2, 512], F32)
                DROW1 = pp.tile([65, 2, 512], F32)
                # out-proj weights (bf16), loaded on the SW DGE queue so
                # the HW DGE queues stay free for x^T chunks
                WO = pp.tile([128, 4, D], BF16)
                WOF = pp.tile([128, 4, D], F32)
                for eb in range(4):
                    nc.gpsimd.dma_start(
                        out=WOF[:, eb, :],
                        in_=wo[eb * 128 : (eb + 1) * 128, :])
                    nc.vector.tensor_copy(WO[:, eb, :], WOF[:, eb, :])

                # ------------ phase 1: projection + V transposes ------
                with tc.tile_pool(name="px", bufs=2) as px, \
                     tc.tile_pool(name="pwq", bufs=1) as pwq, \
                     tc.tile_pool(name="pj", bufs=4, space="PSUM") as pj, \
                     tc.tile_pool(name="ptr", bufs=4, space="PSUM") as ptr:
                    WQ = pwq.tile([128, 8, EH], F32R)
                    QS = [nc.sync, nc.scalar]
                    # x^T streamed by 512-query column chunks (tn-major);
                    # WQ and x interleaved per-kc over both HW DGE queues
                    # so the kc=0 accumulation chain starts ASAP
                    for tn in range(4):
                        XTC = px.tile([128, 8, 512], F32R, tag="xtc")
                        for kc in range(8):
                            if tn == 0:
                                QS[kc % 2].dma_start(
                                    out=WQ[:, kc, :],
                                    in_=wq[kc * 128 : (kc + 1) * 128,
                                           :].bitcast(F32R))
                            QS[(kc + 1 + tn) % 2].dma_start(
                                out=XTC[:, kc, :],
                                in_=xT[kc * 128 : (kc + 1) * 128,
                                       tn * 512 : (tn + 1) * 512
                                       ].bitcast(F32R))
                        for eb in range(4):
                            ps = pj.tile([128, 512], F32, tag="pj")
                            for kc in range(8):
                                nc.tensor.matmul(
                                    ps[:],
                                    WQ[:, kc, eb * 128 : (eb + 1) * 128],
                                    XTC[:, kc, :],
                                    start=(kc == 0), stop=(kc == 7))
                            nc.vector.tensor_copy(
                                QKVT[:, eb, tn * 512 : (tn + 1) * 512],
                                ps[:])
                        # V = transposed qkvT blocks for these 4 t-blocks
                        for jb in range(4):
                            for tb in range(4 * tn, 4 * tn + 4):
                                tp = ptr.tile([128, 128], BF16, tag="tp")
                                nc.tensor.transpose(
                                    tp[:],
                                    QKVT[:, jb, tb * 128 : (tb + 1) * 128],
                                    IDN[:])
                                nc.vector.tensor_copy(
                                    VA[:, tb, 2 * jb : 2 * jb + 2, 0:DH],
                                    tp[:].rearrange(
                                        "p (h d) -> p h d", h=2))

                # ------- phase 2+3 fused: attention + norm + out-proj --
                # PSUM rings: pr1 [128,1024] x2 (scores & out-proj),
                # pr2 [128,512] x4 (PV accumulators & 1/d broadcasts)
                with tc.tile_pool(name="ptp", bufs=14) as ptp, \
                     tc.tile_pool(name="po", bufs=3) as po, \
                     tc.tile_pool(name="pr1", bufs=2, space="PSUM") as pr1, \
                     tc.tile_pool(name="pr2", bufs=4, space="PSUM") as pr2:
                    for th in range(2):
                        for qc in (2 * th, 2 * th + 1):
                            qs = qc * 512
                            nkb = 4 * qc + 4
                            for jb in range(4):
                                qA = QKVT[0:64, jb, :]    # head 2jb
                                qB = QKVT[64:128, jb, :]  # head 2jb+1
                                CA = pr2.tile([128, 512], F32, tag="r2")
                                CB = pr2.tile([128, 512], F32, tag="r2")
                                pts = []
                                for kb in range(nkb):
                                    k0 = kb * 128
                                    q0 = max(k0, qs)
                                    n = qs + 512 - q0
                                    po_ = q0 - qs
                                    sc = pr1.tile([128, 1024], F32,
                                                  tag="r1")
                                    nc.tensor.matmul(
                                        sc[:, 0:n],
                                        qA[:, k0 : k0 + 128],
                                        qA[:, q0 : q0 + n],
                                        start=True, stop=True)
                                    nc.tensor.matmul(
                                        sc[:, 512 : 512 + n],
                                        qB[:, k0 : k0 + 128],
                                        qB[:, q0 : q0 + n],
                                        start=True, stop=True)
                                    pt = ptp.tile([128, 2, n], BF16,
                                                  tag="pt")
                                    nc.scalar.activation(
                                        pt[:],
                                        sc[:].rearrange(
                                            "p (two n) -> p two n", two=2)[
                                            :, :, 0:n],
                                        EXP, scale=0.125)
                                    if k0 >= qs:  # diagonal 128x128 block
                                        nc.vector.tensor_mul(
                                            pt[:, :, 0:128],
                                            pt[:, :, 0:128], MASKB[:])
                                    pts.append((pt, po_, n))
                                for hh, C in ((0, CA), (1, CB)):
                                    for kb, (pt, po_, n) in enumerate(pts):
                                        nc.tensor.matmul(
                                            C[0:65, po_ : po_ + n],
                                            VA[:, kb, 2 * jb + hh, :],
                                            pt[:, hh, :],
                                            start=(kb == 0),
                                            stop=(kb == nkb - 1))
                                for hh, C in ((0, CA), (1, CB)):
                                    nc.vector.tensor_copy(
                                        CTXT[hh * 64 : hh * 64 + 64, jb,
                                             qs : qs + 512],
                                        C[0:64, :])
                                    DR = DROW0 if hh == 0 else DROW1
                                    nc.vector.tensor_copy(
                                        DR[64:65, qc % 2, :], C[64:65, :])
                                    r = (qc % 2) * 8 + jb * 2 + hh
                                    nc.sync.dma_start(
                                        out=DST[th][r : r + 1, :],
                                        in_=DR[64:65, qc % 2, :])
                        with nc.allow_low_precision(
                                reason="f32r recip 1e-4 ok"):
                            nc.vector.reciprocal(RST[th][:], DST[th][:])
                        for qc in (2 * th, 2 * th + 1):
                            for jb in range(4):
                                pr_ = (qc % 2) * 4 + jb
                                BC = pr2.tile([128, 512], F32, tag="r2")
                                nc.tensor.matmul(
                                    BC[:], SEL[th][:, pr_, :],
                                    RST[th][:], start=True, stop=True)
                                for hh in range(2):
                                    dst = CTXT[hh * 64 : hh * 64 + 64, jb,
                                               qc * 512 : qc * 512 + 512]
                                    nc.vector.tensor_mul(
                                        dst, dst,
                                        BC[hh * 64 : hh * 64 + 64, :])
                        # out-projection for this th (overlaps next th's
                        # attention): W_out stationary, CTXT moving
                        for db in range(8):
                            ps = pr1.tile([128, 1024], F32, tag="r1")
                            for eb in range(4):
                                for tcc in range(2):
                                    t0 = (2 * th + tcc) * 512
                                    nc.tensor.matmul(
                                        ps[:, tcc * 512 :
                                           tcc * 512 + 512],
                                        WO[:, eb,
                                           db * 128 : (db + 1) * 128],
                                        CTXT[:, eb, t0 : t0 + 512],
                                        start=(eb == 0), stop=(eb == 3))
                            # evict on ACT: it is idle in the th tails
                            ob = po.tile([128, 1024], F32, tag="ob")
                            nc.scalar.copy(ob[:], ps[:])
                            nc.sync.dma_start(
                                out=outp[db * 128 : (db + 1) * 128,
                                         th * 1024 : (th + 1) * 1024],
                                in_=ob[:])
    nc.compile()
    return nc, {"xT": xT.name, "wq": wq.name, "wo": wo.name,
                "outp": outp.name}


def _get():
    if "nc" not in _CACHE:
        _CACHE["nc"], _CACHE["names"] = _build()
    return _CACHE["nc"], _CACHE["names"]


def _run(x, W_Q, W_out, trace=False):
    from concourse.bass_utils import run_bass_kernel_spmd

    nc, nm = _get()
    in_maps = []
    for c in range(N_CORES):
        b, hg = c // 2, c % 2
        in_maps.append({
            nm["xT"]: np.ascontiguousarray(x[b].T.astype(np.float32)),
            nm["wq"]: np.ascontiguousarray(
                W_Q[:, hg * EH : (hg + 1) * EH].astype(np.float32)),
            nm["wo"]: np.ascontiguousarray(
                W_out[hg * EH : (hg + 1) * EH, :].astype(np.float32)),
        })
    return run_bass_kernel_spmd(
        nc, in_maps, list(range(N_CORES)), trace=trace), nm


def kernel(x, W_Q, W_out, b_out):
    res, nm = _run(np.asarray(x), np.asarray(W_Q), np.asarray(W_out))
    bo = np.asarray(b_out, dtype=np.float32)
    out = np.empty((B, S, D), np.float32)
    for b in range(B):
        # device emits [D, S] partials; transpose back on host
        out[b] = (res.results[2 * b][nm["outp"]]
                  + res.results[2 * b + 1][nm["outp"]]).T + bo
    return out


# revision 20
# speedup vs baseline: 1.0252x; 1.0011x over previous
"""Trainium2 Bass kernel for nn_MultiHeadAttention_39582418600023.

Model (reference bug preserved: Q = K = V = x @ W_Q):
  qkv = x @ W_Q; q,k,v = heads(qkv)
  out = softmax(causal(q k^T) / sqrt(dh)) v  ->  ctx @ W_out + b_out

Sharding (8 cores): data-parallel over batch (4) x tensor-parallel over
head groups (2).  Core c handles batch c//2, heads (c%2)*8 .. +8
(W_Q column-parallel, W_out row-parallel); host sums the two partial
out-projections per batch and adds the bias.

Per-core device kernel (engine budget: ACT=exp only ~150us, PE ~220us,
DVE evictions/masks/norms ~130us):
  1. qkvT[e,t] = W_Qc^T @ x^T (fp32r, x streamed in double-buffered
     512-col chunks over two DMA queues); V via PE transposes into
     ones-augmented VA.
  2. Attention+output fused in one pool block, th(=2 qc)-outer:
     scores for a head pair land in one [128,1024] PSUM tile (ring
     shared with the out-projection PSUM), one exp-ACTIVATE per
     (pair,qc,kb) writes bf16 probs consumed by PV matmuls (VA^T@PT,
     row 64 = softmax denominator).  Causal mask = bf16 upper-tri
     multiply on DVE.  Denominator rows DMA into DST[16,512]; one
     batched reciprocal per th; K=16 one-hot selector matmuls
     broadcast 1/d across partitions; DVE normalizes ctx in place.
  3. Out-projection for a th runs right after its norm, overlapping
     the next th's attention: W_out block stationary (LDWEIGHTS
     reuse), CTXT moving, emits transposed [D,S] partials (host
     transposes back).
"""
import os
import sys

sys.path.insert(0, "/opt/trn_rl_repo")
os.environ.setdefault("MYCRO_LOCAL_CACHE", "1")

import numpy as np

B, S, D = 4, 2048, 1024
NH, DH = 16, 64
EH = 512          # e-columns per core (8 local heads)
NHL = 8           # local heads
N_CORES = 8

_CACHE = {}


def _build():
    import concourse.mybir as mybir
    import concourse.tile as tile
    from concourse import bacc
    from concourse.masks import make_identity, make_upper_triangular

    F32 = mybir.dt.float32
    F32R = mybir.dt.float32r
    BF16 = mybir.dt.bfloat16
    EXP = mybir.ActivationFunctionType.Exp

    nc = bacc.Bacc(None, target_bir_lowering=False, debug=True)
    with tile.TileContext(nc) as tc:
        with tc.tile_pool(name="dram", bufs=1, space="DRAM") as dram:
            xT = dram.tile([D, S], F32, kind="ExternalInput")      # x[b].T
            wq = dram.tile([D, EH], F32, kind="ExternalInput")     # W_Q cols
            wo = dram.tile([EH, D], F32, kind="ExternalInput")     # W_out rows
            # partial out, TRANSPOSED [D, S]; host transposes back
            outp = dram.tile([D, S], F32, kind="ExternalOutput")

            with tc.tile_pool(name="persist", bufs=1) as pp:
                # qkvT: [e-block 128, eb, t], bf16 (scores operands)
                QKVT = pp.tile([128, 4, S], BF16)
                # ones-augmented V (bf16): [t%128, tb, h, 0:64]=V, [..,64]=1
                VA = pp.tile([128, 16, NHL, DH + 1], BF16)
                # unnormalized ctxT (bf16), same layout as QKVT
                CTXT = pp.tile([128, 4, S], BF16)
                IDN = pp.tile([128, 128], BF16)
                MASKF = pp.tile([128, 128], F32)  # 1 on i<=j else 0
                MASKB = pp.tile([128, 2, 128], BF16)
                make_upper_triangular(nc, MASKF[:], val=1.0, diag=True)
                make_identity(nc, IDN[:])
                nc.vector.tensor_copy(MASKB[:, 0, :], MASKF[:])
                nc.vector.tensor_copy(MASKB[:, 1, :], MASKF[:])
                nc.vector.memset(VA[:, :, :, DH : DH + 1], 1.0)
                # denominator staging per qc (4 jb x 2 hh rows):
                # row r = jb*2 + hh, partitions 0..7
                DST = [pp.tile([8, 512], F32, name=f"DST{t}")
                       for t in range(4)]
                RST = [pp.tile([8, 512], F32R, name=f"RST{t}")
                       for t in range(4)]
                # one-hot selector [8, jb, 128]: SEL[k, jb, j] = 1 iff
                # k == jb*2 + (j>=64); BC = SEL^T @ RST (same every qc)
                SELF_ = pp.tile([8, 4, 128], F32)
                SELQ = pp.tile([8, 4, 128], F32R)
                nc.gpsimd.memset(SELF_[:], 1.0)
                for jb in range(4):
                    for hh in range(2):
                        ks = jb * 2 + hh
                        nc.gpsimd.affine_select(
                            out=SELF_[:, jb, hh * 64 : hh * 64 + 64],
                            in_=SELF_[:, jb, hh * 64 : hh * 64 + 64],
                            compare_op=mybir.AluOpType.is_equal,
                            fill=0.0, base=-ks,
                            pattern=[[0, 64]], channel_multiplier=1)
                nc.vector.tensor_copy(SELQ[:], SELF_[:])
                # denom rows staged at partition 64 (DVE evict from PSUM;
                # DMA cannot read PSUM)
                DROW0 = pp.tile([65, 2, 512], F32)
                DROW1 = pp.tile([65, 2, 512], F32)
                # out-proj weights (bf16), loaded on the SW DGE queue so
                # the HW DGE queues stay free for x^T chunks
                WO = pp.tile([128, 4, D], BF16)
                WOF = pp.tile([128, 4, D], F32)
                for eb in range(4):
                    nc.gpsimd.dma_start(
                        out=WOF[:, eb, :],
                        in_=wo[eb * 128 : (eb + 1) * 128, :])
                    nc.vector.tensor_copy(WO[:, eb, :], WOF[:, eb, :])

                # ------------ phase 1: projection + V transposes ------
                with tc.tile_pool(name="px", bufs=2) as px, \
                     tc.tile_pool(name="pwq", bufs=1) as pwq, \
                     tc.tile_pool(name="pj", bufs=4, space="PSUM") as pj, \
                     tc.tile_pool(name="ptr", bufs=4, space="PSUM") as ptr:
                    WQ = pwq.tile([128, 8, EH], F32R)
                    QS = [nc.sync, nc.scalar]
                    # x^T streamed by 512-query column chunks (tn-major);
                    # WQ and x interleaved per-kc over both HW DGE queues
                    # so the kc=0 accumulation chain starts ASAP
                    for tn in range(4):
                        XTC = px.tile([128, 8, 512], F32R, tag="xtc")
                        for kc in range(8):
                            if tn == 0:
                                QS[kc % 2].dma_start(
                                    out=WQ[:, kc, :],
                                    in_=wq[kc * 128 : (kc + 1) * 128,
                                           :].bitcast(F32R))
                            QS[(kc + 1 + tn) % 2].dma_start(
                                out=XTC[:, kc, :],
                                in_=xT[kc * 128 : (kc + 1) * 128,
                                       tn * 512 : (tn + 1) * 512
                                       ].bitcast(F32R))
                        for eb in range(4):
                            ps = pj.tile([128, 512], F32, tag="pj")
                            for kc in range(8):
                                nc.tensor.matmul(
                                    ps[:],
                                    WQ[:, kc, eb * 128 : (eb + 1) * 128],
                                    XTC[:, kc, :],
                                    start=(kc == 0), stop=(kc == 7))
                            nc.vector.tensor_copy(
                                QKVT[:, eb, tn * 512 : (tn + 1) * 512],
                                ps[:])
                        # V = transposed qkvT blocks for these 4 t-blocks
                        for jb in range(4):
                            for tb in range(4 * tn, 4 * tn + 4):
                                tp = ptr.tile([128, 128], BF16, tag="tp")
                                nc.tensor.transpose(
                                    tp[:],
                                    QKVT[:, jb, tb * 128 : (tb + 1) * 128],
                                    IDN[:])
                                nc.vector.tensor_copy(
                                    VA[:, tb, 2 * jb : 2 * jb + 2, 0:DH],
                                    tp[:].rearrange(
                                        "p (h d) -> p h d", h=2))

                # ------- phase 2+3 fused: attention + norm + out-proj --
                # PSUM rings: pr1 [128,1024] x2 (scores & out-proj),
                # pr2 [128,512] x4 (PV accumulators & 1/d broadcasts)
                with tc.tile_pool(name="ptp", bufs=14) as ptp, \
                     tc.tile_pool(name="po", bufs=4) as po, \
                     tc.tile_pool(name="pr1", bufs=2, space="PSUM") as pr1, \
                     tc.tile_pool(name="pr2", bufs=4, space="PSUM") as pr2:
                    for qc in range(4):
                        qs = qc * 512
                        nkb = 4 * qc + 4
                        for jb in range(4):
                            qA = QKVT[0:64, jb, :]    # head 2jb
                            qB = QKVT[64:128, jb, :]  # head 2jb+1
                            CA = pr2.tile([128, 512], F32, tag="r2")
                            CB = pr2.tile([128, 512], F32, tag="r2")
                            pts = []
                            for kb in range(nkb):
                                k0 = kb * 128
                                q0 = max(k0, qs)
                                n = qs + 512 - q0
                                po_ = q0 - qs
                                sc = pr1.tile([128, 1024], F32, tag="r1")
                                nc.tensor.matmul(
                                    sc[:, 0:n],
                                    qA[:, k0 : k0 + 128],
                                    qA[:, q0 : q0 + n],
                                    start=True, stop=True)
                                nc.tensor.matmul(
                                    sc[:, 512 : 512 + n],
                                    qB[:, k0 : k0 + 128],
                                    qB[:, q0 : q0 + n],
                                    start=True, stop=True)
                                pt = ptp.tile([128, 2, n], BF16,
                                              tag="pt")
                                nc.scalar.activation(
                                    pt[:],
                                    sc[:].rearrange(
                                        "p (two n) -> p two n", two=2)[
                                        :, :, 0:n],
                                    EXP, scale=0.125)
                                if k0 >= qs:  # diagonal 128x128 block
                                    nc.vector.tensor_mul(
                                        pt[:, :, 0:128],
                                        pt[:, :, 0:128], MASKB[:])
                                pts.append((pt, po_, n))
                            for hh, C in ((0, CA), (1, CB)):
                                for kb, (pt, po_, n) in enumerate(pts):
                                    nc.tensor.matmul(
                                        C[0:65, po_ : po_ + n],
                                        VA[:, kb, 2 * jb + hh, :],
                                        pt[:, hh, :],
                                        start=(kb == 0),
                                        stop=(kb == nkb - 1))
                            for hh, C in ((0, CA), (1, CB)):
                                nc.vector.tensor_copy(
                                    CTXT[hh * 64 : hh * 64 + 64, jb,
                                         qs : qs + 512],
                                    C[0:64, :])
                                DR = DROW0 if hh == 0 else DROW1
                                nc.vector.tensor_copy(
                                    DR[64:65, qc % 2, :], C[64:65, :])
                                nc.sync.dma_start(
                                    out=DST[qc][2 * jb + hh :
                                                2 * jb + hh + 1, :],
                                    in_=DR[64:65, qc % 2, :])
                        # ---- per-qc finish: recip, broadcast, norm,
                        # out-projection (ACT drains its EXP backlog
                        # meanwhile; tail after last EXP stays short) ----
                        with nc.allow_low_precision(
                                reason="f32r recip 1e-4 ok"):
                            nc.vector.reciprocal(RST[qc][:], DST[qc][:])
                        for jb in range(4):
                            BC = pr2.tile([128, 512], F32, tag="r2")
                            nc.tensor.matmul(
                                BC[:], SELQ[:, jb, :],
                                RST[qc][:], start=True, stop=True)
                            for hh in range(2):
                                dst = CTXT[hh * 64 : hh * 64 + 64, jb,
                                           qs : qs + 512]
                                nc.vector.tensor_mul(
                                    dst, dst,
                                    BC[hh * 64 : hh * 64 + 64, :])
                        for db in range(8):
                            ps = pr2.tile([128, 512], F32, tag="r2")
                            for eb in range(4):
                                nc.tensor.matmul(
                                    ps[:],
                                    WO[:, eb, db * 128 : (db + 1) * 128],
                                    CTXT[:, eb, qs : qs + 512],
                                    start=(eb == 0), stop=(eb == 3))
                            ob = po.tile([128, 512], F32, tag="ob")
                            # evict on ACT except where ACT saturates
                            # (qc==2 overlaps qc3's exp stream)
                            if qc == 2:
                                nc.vector.tensor_copy(ob[:], ps[:])
                            else:
                                nc.scalar.copy(ob[:], ps[:])
                            nc.sync.dma_start(
                                out=outp[db * 128 : (db + 1) * 128,
                                         qs : qs + 512],
                                in_=ob[:])
    nc.compile()
    return nc, {"xT": xT.name, "wq": wq.name, "wo": wo.name,
                "outp": outp.name}


def _get():
    if "nc" not in _CACHE:
        _CACHE["nc"], _CACHE["names"] = _build()
    return _CACHE["nc"], _CACHE["names"]


def _run(x, W_Q, W_out, trace=False):
    from concourse.bass_utils import run_bass_kernel_spmd

    nc, nm = _get()
    in_maps = []
    for c in range(N_CORES):
        b, hg = c // 2, c % 2
        in_maps.append({
            nm["xT"]: np.ascontiguousarray(x[b].T.astype(np.float32)),
            nm["wq"]: np.ascontiguousarray(
                W_Q[:, hg * EH : (hg + 1) * EH].astype(np.float32)),
            nm["wo"]: np.ascontiguousarray(
                W_out[hg * EH : (hg + 1) * EH, :].astype(np.float32)),
        })
    return run_bass_kernel_spmd(
        nc, in_maps, list(range(N_CORES)), trace=trace), nm


def kernel(x, W_Q, W_out, b_out):
    res, nm = _run(np.asarray(x), np.asarray(W_Q), np.asarray(W_out))
    bo = np.asarray(b_out, dtype=np.float32)
    out = np.empty((B, S, D), np.float32)
    for b in range(B):
        # device emits [D, S] partials; transpose back on host
        out[b] = (res.results[2 * b][nm["outp"]]
                  + res.results[2 * b + 1][nm["outp"]]).T + bo
    return out


# revision 29
# speedup vs baseline: 1.0593x; 1.0332x over previous
"""Trainium2 Bass kernel for nn_MultiHeadAttention_39582418600023.

Model (reference bug preserved: Q = K = V = x @ W_Q):
  qkv = x @ W_Q; q,k,v = heads(qkv)
  out = softmax(causal(q k^T) / sqrt(dh)) v  ->  ctx @ W_out + b_out

Sharding (8 cores): data-parallel over batch (4) x tensor-parallel over
head groups (2).  Core c handles batch c//2, heads (c%2)*8 .. +8
(W_Q column-parallel, W_out row-parallel); host sums the two partial
out-projections per batch and adds the bias.

Per-core device kernel (engine budget: ACT=exp only ~150us, PE ~220us,
DVE evictions/masks/norms ~130us):
  1. qkvT[e,t] = W_Qc^T @ x^T (fp32r, x streamed in double-buffered
     512-col chunks over two DMA queues); V via PE transposes into
     ones-augmented VA.
  2. Attention+output fused in one pool block, th(=2 qc)-outer:
     scores for a head pair land in one [128,1024] PSUM tile (ring
     shared with the out-projection PSUM), one exp-ACTIVATE per
     (pair,qc,kb) writes bf16 probs consumed by PV matmuls (VA^T@PT,
     row 64 = softmax denominator).  Causal mask = bf16 upper-tri
     multiply on DVE.  Denominator rows DMA into DST[16,512]; one
     batched reciprocal per th; K=16 one-hot selector matmuls
     broadcast 1/d across partitions; DVE normalizes ctx in place.
  3. Out-projection for a th runs right after its norm, overlapping
     the next th's attention: W_out block stationary (LDWEIGHTS
     reuse), CTXT moving, emits transposed [D,S] partials (host
     transposes back).
"""
import os
import sys

sys.path.insert(0, "/opt/trn_rl_repo")
os.environ.setdefault("MYCRO_LOCAL_CACHE", "1")

import numpy as np

B, S, D = 4, 2048, 1024
NH, DH = 16, 64
EH = 512          # e-columns per core (8 local heads)
NHL = 8           # local heads
N_CORES = 8

_CACHE = {}


def _build():
    import concourse.mybir as mybir
    import concourse.tile as tile
    from concourse import bacc
    from concourse.masks import make_identity, make_upper_triangular

    F32 = mybir.dt.float32
    F32R = mybir.dt.float32r
    BF16 = mybir.dt.bfloat16
    EXP = mybir.ActivationFunctionType.Exp

    nc = bacc.Bacc(None, target_bir_lowering=False, debug=True)
    with tile.TileContext(nc) as tc:
        with tc.tile_pool(name="dram", bufs=1, space="DRAM") as dram:
            xT = dram.tile([D, S], F32, kind="ExternalInput")      # x[b].T
            wq = dram.tile([D, EH], F32, kind="ExternalInput")     # W_Q cols
            wo = dram.tile([EH, D], F32, kind="ExternalInput")     # W_out rows
            # partial out, TRANSPOSED [D, S]; host transposes back
            outp = dram.tile([D, S], F32, kind="ExternalOutput")

            with tc.tile_pool(name="persist", bufs=1) as pp:
                # qkvT: [e-block 128, eb, t], bf16 (scores operands)
                QKVT = pp.tile([128, 4, S], BF16)
                # ones-augmented V (bf16): [t%128, tb, h, 0:64]=V, [..,64]=1
                VA = pp.tile([128, 16, NHL, DH + 1], BF16)
                # unnormalized ctxT (bf16), same layout as QKVT
                CTXT = pp.tile([128, 4, S], BF16)
                IDN = pp.tile([128, 128], BF16)
                MASKF = pp.tile([128, 128], F32)  # 1 on i<=j else 0
                MASKB = pp.tile([128, 2, 128], BF16)
                make_upper_triangular(nc, MASKF[:], val=1.0, diag=True)
                make_identity(nc, IDN[:])
                nc.vector.tensor_copy(MASKB[:, 0, :], MASKF[:])
                nc.vector.tensor_copy(MASKB[:, 1, :], MASKF[:])
                nc.vector.memset(VA[:, :, :, DH : DH + 1], 1.0)
                # denominator staging per qc (4 jb x 2 hh rows):
                # row r = jb*2 + hh, partitions 0..7
                DST = [pp.tile([8, 512], F32, name=f"DST{t}")
                       for t in range(4)]
                RST = [pp.tile([8, 512], F32, name=f"RST{t}")
                       for t in range(4)]
                RSTR = [pp.tile([8, 512], F32R, name=f"RSTR{t}")
                        for t in range(4)]
                # one-hot selector [8, jb, 128]: SEL[k, jb, j] = 1 iff
                # k == jb*2 + (j>=64); BC = SEL^T @ RST (same every qc)
                SELF_ = pp.tile([8, 4, 128], F32)
                SELQ = pp.tile([8, 4, 128], F32R)
                nc.gpsimd.memset(SELF_[:], 1.0)
                for jb in range(4):
                    for hh in range(2):
                        ks = jb * 2 + hh
                        nc.gpsimd.affine_select(
                            out=SELF_[:, jb, hh * 64 : hh * 64 + 64],
                            in_=SELF_[:, jb, hh * 64 : hh * 64 + 64],
                            compare_op=mybir.AluOpType.is_equal,
                            fill=0.0, base=-ks,
                            pattern=[[0, 64]], channel_multiplier=1)
                nc.vector.tensor_copy(SELQ[:], SELF_[:])
                # denom rows staged at partition 64 (DVE evict from PSUM;
                # DMA cannot read PSUM)
                DROW0 = pp.tile([65, 2, 512], F32)
                DROW1 = pp.tile([65, 2, 512], F32)
                # out-proj weights (bf16), loaded on the SW DGE queue so
                # the HW DGE queues stay free for x^T chunks
                WO = pp.tile([128, 4, D], BF16)
                WOF = pp.tile([128, 4, D], F32)
                for eb in range(4):
                    nc.gpsimd.dma_start(
                        out=WOF[:, eb, :],
                        in_=wo[eb * 128 : (eb + 1) * 128, :])
                    nc.vector.tensor_copy(WO[:, eb, :], WOF[:, eb, :])

                # ------------ phase 1: projection + V transposes ------
                with tc.tile_pool(name="px", bufs=2) as px, \
                     tc.tile_pool(name="pwq", bufs=1) as pwq, \
                     tc.tile_pool(name="pj", bufs=4, space="PSUM") as pj, \
                     tc.tile_pool(name="ptr", bufs=4, space="PSUM") as ptr:
                    WQ = pwq.tile([128, 8, EH], F32R)
                    QS = [nc.sync, nc.scalar]
                    # x^T streamed by 512-query column chunks (tn-major);
                    # WQ and x interleaved per-kc over both HW DGE queues
                    # so the kc=0 accumulation chain starts ASAP
                    for tn in range(4):
                        XTC = px.tile([128, 8, 512], F32R, tag="xtc")
                        for kc in range(8):
                            if tn == 0:
                                QS[kc % 2].dma_start(
                                    out=WQ[:, kc, :],
                                    in_=wq[kc * 128 : (kc + 1) * 128,
                                           :].bitcast(F32R))
                            QS[(kc + 1 + tn) % 2].dma_start(
                                out=XTC[:, kc, :],
                                in_=xT[kc * 128 : (kc + 1) * 128,
                                       tn * 512 : (tn + 1) * 512
                                       ].bitcast(F32R))
                        for eb in range(4):
                            ps = pj.tile([128, 512], F32, tag="pj")
                            for kc in range(8):
                                nc.tensor.matmul(
                                    ps[:],
                                    WQ[:, kc, eb * 128 : (eb + 1) * 128],
                                    XTC[:, kc, :],
                                    start=(kc == 0), stop=(kc == 7))
                            nc.vector.tensor_copy(
                                QKVT[:, eb, tn * 512 : (tn + 1) * 512],
                                ps[:])
                        # V = transposed qkvT blocks for these 4 t-blocks
                        for jb in range(4):
                            for tb in range(4 * tn, 4 * tn + 4):
                                tp = ptr.tile([128, 128], BF16, tag="tp")
                                nc.tensor.transpose(
                                    tp[:],
                                    QKVT[:, jb, tb * 128 : (tb + 1) * 128],
                                    IDN[:])
                                nc.vector.tensor_copy(
                                    VA[:, tb, 2 * jb : 2 * jb + 2, 0:DH],
                                    tp[:].rearrange(
                                        "p (h d) -> p h d", h=2))

                # ------- phase 2+3 fused: attention + norm + out-proj --
                # PSUM rings: pr1 [128,1024] x2 (scores & out-proj),
                # pr2 [128,512] x4 (PV accumulators & 1/d broadcasts)
                with tc.tile_pool(name="ptp", bufs=14) as ptp, \
                     tc.tile_pool(name="po", bufs=4) as po, \
                     tc.tile_pool(name="pr1", bufs=2, space="PSUM") as pr1, \
                     tc.tile_pool(name="pr2", bufs=4, space="PSUM") as pr2:

                    def emit_outproj(qcx, on_dve):
                        qsx = qcx * 512
                        for db in range(8):
                            ps = pr2.tile([128, 512], F32, tag="r2",
                                          name=f"ps{qcx}_{db}")
                            for eb in range(4):
                                nc.tensor.matmul(
                                    ps[:],
                                    WO[:, eb, db * 128 : (db + 1) * 128],
                                    CTXT[:, eb, qsx : qsx + 512],
                                    start=(eb == 0), stop=(eb == 3))
                            ob = po.tile([128, 512], F32, tag="ob",
                                         name=f"ob{qcx}_{db}")
                            # ACT evicts where it has slack; DVE where
                            # ACT saturates (inside qc3)
                            if on_dve:
                                nc.vector.tensor_copy(ob[:], ps[:])
                            else:
                                nc.scalar.copy(ob[:], ps[:])
                            nc.sync.dma_start(
                                out=outp[db * 128 : (db + 1) * 128,
                                         qsx : qsx + 512],
                                in_=ob[:])

                    for qc in range(4):
                        qs = qc * 512
                        nkb = 4 * qc + 4
                        for jb in range(4):
                            qA = QKVT[0:64, jb, :]    # head 2jb
                            qB = QKVT[64:128, jb, :]  # head 2jb+1
                            CA = pr2.tile([128, 512], F32, tag="r2")
                            CB = pr2.tile([128, 512], F32, tag="r2")
                            pts = []
                            for kb in range(nkb):
                                k0 = kb * 128
                                q0 = max(k0, qs)
                                n = qs + 512 - q0
                                po_ = q0 - qs
                                sc = pr1.tile([128, 1024], F32, tag="r1")
                                nc.tensor.matmul(
                                    sc[:, 0:n],
                                    qA[:, k0 : k0 + 128],
                                    qA[:, q0 : q0 + n],
                                    start=True, stop=True)
                                nc.tensor.matmul(
                                    sc[:, 512 : 512 + n],
                                    qB[:, k0 : k0 + 128],
                                    qB[:, q0 : q0 + n],
                                    start=True, stop=True)
                                pt = ptp.tile([128, 2, n], BF16,
                                              tag="pt")
                                nc.scalar.activation(
                                    pt[:],
                                    sc[:].rearrange(
                                        "p (two n) -> p two n", two=2)[
                                        :, :, 0:n],
                                    EXP, scale=0.125)
                                if k0 >= qs:  # diagonal 128x128 block
                                    nc.vector.tensor_mul(
                                        pt[:, :, 0:128],
                                        pt[:, :, 0:128], MASKB[:])
                                pts.append((pt, po_, n))
                            for hh, C in ((0, CA), (1, CB)):
                                for kb, (pt, po_, n) in enumerate(pts):
                                    nc.tensor.matmul(
                                        C[0:65, po_ : po_ + n],
                                        VA[:, kb, 2 * jb + hh, :],
                                        pt[:, hh, :],
                                        start=(kb == 0),
                                        stop=(kb == nkb - 1))
                            # denom rows first: they gate the recip/BC
                            # chain; bulky ctx evicts follow
                            for hh, C in ((0, CA), (1, CB)):
                                DR = DROW0 if hh == 0 else DROW1
                                nc.vector.tensor_copy(
                                    DR[64:65, qc % 2, :], C[64:65, :])
                                nc.sync.dma_start(
                                    out=DST[qc][2 * jb + hh :
                                                2 * jb + hh + 1, :],
                                    in_=DR[64:65, qc % 2, :])
                            for hh, C in ((0, CA), (1, CB)):
                                nc.vector.tensor_copy(
                                    CTXT[hh * 64 : hh * 64 + 64, jb,
                                         qs : qs + 512],
                                    C[0:64, :])
                            if qc == 3 and jb == 2:
                                emit_outproj(2, on_dve=True)
                        # ---- per-qc finish: recip, broadcast, norm,
                        # out-projection (ACT drains its EXP backlog
                        # meanwhile; tail after last EXP stays short) ----
                        with nc.allow_low_precision(
                                reason="~18-bit recip plenty for softmax"):
                            nc.vector.reciprocal_approx_fast(
                                RST[qc][:], DST[qc][:])
                            nc.vector.tensor_copy(RSTR[qc][:], RST[qc][:])
                        for jb in range(4):
                            BC = pr2.tile([128, 512], F32, tag="r2")
                            nc.tensor.matmul(
                                BC[:], SELQ[:, jb, :],
                                RSTR[qc][:], start=True, stop=True)
                            for hh in range(2):
                                dst = CTXT[hh * 64 : hh * 64 + 64, jb,
                                           qs : qs + 512]
                                nc.vector.tensor_mul(
                                    dst, dst,
                                    BC[hh * 64 : hh * 64 + 64, :])

                        # qc2's out-proj is deferred into qc3's attention
                        # (emitted after qc3/jb2) so the exp backlog covers
                        # its PE time; others run in place
                        if qc != 2:
                            emit_outproj(qc, on_dve=False)
    nc.compile()
    return nc, {"xT": xT.name, "wq": wq.name, "wo": wo.name,
                "outp": outp.name}


def _get():
    if "nc" not in _CACHE:
        _CACHE["nc"], _CACHE["names"] = _build()
    return _CACHE["nc"], _CACHE["names"]


def _run(x, W_Q, W_out, trace=False):
    from concourse.bass_utils import run_bass_kernel_spmd

    nc, nm = _get()
    in_maps = []
    for c in range(N_CORES):
        b, hg = c // 2, c % 2
        in_maps.append({
            nm["xT"]: np.ascontiguousarray(x[b].T.astype(np.float32)),
            nm["wq"]: np.ascontiguousarray(
                W_Q[:, hg * EH : (hg + 1) * EH].astype(np.float32)),
            nm["wo"]: np.ascontiguousarray(
                W_out[hg * EH : (hg + 1) * EH, :].astype(np.float32)),
        })
    return run_bass_kernel_spmd(
        nc, in_maps, list(range(N_CORES)), trace=trace), nm


def kernel(x, W_Q, W_out, b_out):
    res, nm = _run(np.asarray(x), np.asarray(W_Q), np.asarray(W_out))
    bo = np.asarray(b_out, dtype=np.float32)
    out = np.empty((B, S, D), np.float32)
    for b in range(B):
        # device emits [D, S] partials; transpose back on host
        out[b] = (res.results[2 * b][nm["outp"]]
                  + res.results[2 * b + 1][nm["outp"]]).T + bo
    return out
